# revision 14
# baseline (speedup 1.0000x reference)
"""AdaptiveEdgeSmoothing Trainium2 kernel.

Reference semantics (per sample, 1024x1024 f32 image):
    edges     = |conv3x3(mask, LAPLACIAN)|          (SAME zero pad)
    edge_mask = edges > 0.5*edge_sensitivity
    sm        = mask*(1-bf) + box5(mask)/25*bf,  bf = blur_strength/3
    result    = where(edge_mask, sm, mask)
    out       = (result > final_threshold).astype(f32)

Strategy: B=16 samples sharded 2-per-core across 8 NeuronCores (pure data
parallel).  Per core, each image is processed in 9 row-tiles (rows on
partitions, cols on the free axis).  All convolution arithmetic runs on the
TensorEngine as banded fp32r matmuls over column-shifted rhs views of
zero-margined SBUF blocks:
    PSUM1 = 9x - box3(x)            (3 accumulating passes; the Laplacian)
    PSUM2 = (bf/25)*box5(x)+(1-bf)x (5 passes; the smoothed value)
Vertical band weights (incl. SAME-pad clipping and the per-sample bf
scaling) are precomputed in numpy and DMA'd in.  Halo rows are parked at
spare partitions so output rows start at partition 0 on every operand.
Row-tiles are packed side by side in the free axis of big per-image SBUF
buffers so that loads and stores are a few >1MiB SWDGE (gpsimd) DMAs,
which spread across all 16 SDMA engines (HWDGE transfers chunk
32-partitions-per-engine and cap at ~4 engines).  Elementwise tail: ACT
computes Relu(|lap| - thr) as an edge mask (nonzero = edge), DVE
copy_predicated overwrites a copy of x with sm where masked, then one
is_gt against final_threshold writes the packed output block.
"""

import sys

if '/opt/trn_rl_repo' not in sys.path:
    sys.path.insert(0, '/opt/trn_rl_repo')

import numpy as np

import concourse.bass as bass
import concourse.bacc as bacc
import concourse.bass_utils as bass_utils
import concourse.mybir as mybir
from concourse.tile import TileContext, add_dep_helper
from concourse.bass_utils import run_bass_kernel_spmd

# Enable walrus's LDWEIGHTS optimization for this kernel's compile:
# consecutive matmuls sharing a stationary operand skip redundant weight
# loads.  (The flag is hardcoded off in bir_verify_and_optimise.)
if not getattr(bass_utils, "_ldw_opt_patched", False):
    _orig_run_command = bass_utils.run_command

    def _run_command_ldw(argv, **kwargs):
        if isinstance(argv, list):
            argv = ["--enable-ldw-opt=true" if a == "--enable-ldw-opt=false"
                    else a for a in argv]
        return _orig_run_command(argv, **kwargs)

    bass_utils.run_command = _run_command_ldw
    bass_utils._ldw_opt_patched = True

H = W = 1024
N_CORES = 8
IMGS_PER_CORE = 2
F32 = mybir.dt.float32
F32R = mybir.dt.float32r
XP = 1028  # padded block pitch (2-col zero margins each side)

# tile geometry: (out_row_start, n_out, K_data, halo_partition_base, var)
# partitions [0, K_data) hold rows [s, s+K_data); partitions
# [halo_base, halo_base+2) hold rows [s-2, s).
TILES = [(124 * t, 124, 126, 126, (0 if t == 0 else 1)) for t in range(8)]
TILES.append((992, 32, 32, 32, 2))


def _band_templates():
    """Per variant: (V3, V5, I) as [128,128] f32, plus (K_total, nout)."""
    out = []
    for var in range(3):
        s, nout, kd, hb, _ = TILES[0 if var == 0 else (1 if var == 1 else 8)]
        v3 = np.zeros((128, 128), np.float32)
        v5 = np.zeros((128, 128), np.float32)
        ident = np.zeros((128, 128), np.float32)
        for k in range(kd):
            for p in range(nout):
                d = k - p
                if abs(d) <= 1:
                    v3[k, p] = 1.0
                if abs(d) <= 2:
                    v5[k, p] = 1.0
                if d == 0:
                    ident[k, p] = 1.0
        if var != 0:  # top halo rows: partition hb+j holds row s-2+j
            for j in range(2):
                for p in range(nout):
                    d = (j - 2) - p
                    if abs(d) <= 1:
                        v3[hb + j, p] = 1.0
                    if abs(d) <= 2:
                        v5[hb + j, p] = 1.0
        k_tot = 128 if var != 2 else 34
        out.append((v3, v5, ident, k_tot, nout))
    return out


_TEMPLATES = _band_templates()

_compiled = None
last_results = None


def _margin_memsets(nc, blk, nblocks):
    """Zero the 2-col margins of every 1028-wide block in `blk`."""
    nc.vector.memset(blk[:, 0:2].bitcast(F32), 0)
    if nblocks > 1:
        # right margin of block t + left margin of block t+1 are contiguous
        spans = blk[:, 1026:1026 + (nblocks - 1) * XP].rearrange(
            "p (t c) -> p t c", c=XP)[:, :, 0:4]
        nc.vector.memset(spans.bitcast(F32), 0)
    nc.vector.memset(
        blk[:, nblocks * XP - 2:nblocks * XP].bitcast(F32), 0)


def _build():
    nc = bacc.Bacc("TRN2", target_bir_lowering=False, debug=False,
                   num_devices=N_CORES)
    x = nc.dram_tensor("x", [IMGS_PER_CORE, H, W], F32R,
                       kind="ExternalInput")
    w3p = nc.dram_tensor("w3p", [128, 3 * 2 * 128], F32R,
                         kind="ExternalInput").ap()
    w5p = nc.dram_tensor("w5p", [128, IMGS_PER_CORE * 3 * 2 * 128], F32R,
                         kind="ExternalInput").ap()
    negthr = nc.dram_tensor("negthr", [IMGS_PER_CORE, 128, 1], F32,
                            kind="ExternalInput").ap()
    ft = nc.dram_tensor("ft", [IMGS_PER_CORE, 128, 1], F32,
                        kind="ExternalInput").ap()
    y = nc.dram_tensor("out", [IMGS_PER_CORE, H, W], F32,
                       kind="ExternalOutput")

    def xdma(img, out_ap, row0, nrows, ntiles):
        """DRAM read AP: partition p, block t -> image row row0 + 124t + p."""
        return nc.gpsimd.dma_start(
            out=out_ap,
            in_=bass.AP(x, img * H * W + row0 * W,
                        [[W, nrows], [124 * W, ntiles], [1, W]]))

    with TileContext(nc) as tc:
        with (
            tc.tile_pool(name="wpool", bufs=1) as wpool,
            tc.tile_pool(name="spool", bufs=1) as spool,
            tc.tile_pool(name="xpool", bufs=1) as xpool,
            tc.tile_pool(name="p1pool", bufs=2, space="PSUM") as p1pool,
            tc.tile_pool(name="p2pool", bufs=2, space="PSUM") as p2pool,
            tc.tile_pool(name="apool", bufs=2) as apool,
            tc.tile_pool(name="empool", bufs=2) as empool,
            tc.tile_pool(name="vpool", bufs=2) as vpool,
            tc.tile_pool(name="opool", bufs=1) as opool,
        ):
            # --- one-time loads: weights + per-image scalars -------------
            # (HWDGE so they don't queue ahead of image data on the SWDGE
            # rings)
            w3all = wpool.tile([128, 3 * 2 * 128], F32R, tag="w3all")
            nc.sync.dma_start(out=w3all[:], in_=w3p)
            w5all = wpool.tile([128, IMGS_PER_CORE * 3 * 2 * 128], F32R,
                               tag="w5all")
            nc.scalar.dma_start(out=w5all[:], in_=w5p)

            def w3_ap(v, sc):
                return w3all[:, (v * 2 + sc) * 128:(v * 2 + sc) * 128 + 128]

            def w5_ap(img, v, sc):
                base = ((img * 3 + v) * 2 + sc) * 128
                return w5all[:, base:base + 128]

            sc_t = []
            for img in range(IMGS_PER_CORE):
                nt = spool.tile([128, 1], F32, tag=f"nt{img}")
                f = spool.tile([128, 1], F32, tag=f"ft{img}")
                nc.sync.dma_start(out=nt[:], in_=negthr[img])
                nc.sync.dma_start(out=f[:], in_=ft[img])
                sc_t.append((nt, f))

            def emit_loads(img):
                """Packed per-image loads; returns (xa3, xb3, first_inst)."""
                xa = xpool.tile([128, 4 * XP], F32R, tag=f"xa{img}")
                xb = xpool.tile([128, 5 * XP], F32R, tag=f"xb{img}")
                _margin_memsets(nc, xa, 4)
                _margin_memsets(nc, xb, 5)
                xa3 = xa[:, :].rearrange("p (t c) -> p t c", c=XP)
                xb3 = xb[:, :].rearrange("p (t c) -> p t c", c=XP)
                # main rows, blocks 0..3 and 4..7
                ld0 = xdma(img, xa3[0:126, 0:4, 2:1026], 0, 126, 4)
                # halo rows (rows s-2, s-1 per block)
                xdma(img, xa3[126:128, 0:1, 2:1026], 0, 2, 1)  # t0 dummy
                xdma(img, xa3[126:128, 1:4, 2:1026], 124 * 1 - 2, 2, 3)
                xdma(img, xb3[0:126, 0:4, 2:1026], 124 * 4, 126, 4)
                xdma(img, xb3[126:128, 0:4, 2:1026], 124 * 4 - 2, 2, 4)
                # last tile (block 8 of xb): rows 992..1023 + halo 990..991
                nc.gpsimd.dma_start(out=xb3[0:32, 4, 2:1026],
                                    in_=x.ap()[img, 992:1024, :])
                nc.gpsimd.dma_start(out=xb3[32:34, 4, 2:1026],
                                    in_=x.ap()[img, 990:992, :])
                return xa3, xb3, ld0

            # --- main loop ----------------------------------------------
            xab = {0: emit_loads(0)}
            first_mm = None
            for img in range(IMGS_PER_CORE):
                nt_ap, ft_ap = sc_t[img]
                xa3, xb3, _ = xab[img]
                oa = opool.tile([128, 4 * 1024], F32, tag=f"oa{img}")
                ob = opool.tile([128, 5 * 1024], F32, tag=f"ob{img}")
                oa3 = oa[:, :].rearrange("p (t c) -> p t c", c=1024)
                ob3 = ob[:, :].rearrange("p (t c) -> p t c", c=1024)
                for t, (s, nout, kd, hb, var) in enumerate(TILES):
                    k_tot = _TEMPLATES[var][3]
                    xt3, blk = (xa3, t) if t < 4 else (xb3, t - 4)

                    p1 = p1pool.tile([128, 1024], F32, tag="p1")
                    p2 = p2pool.tile([128, 1024], F32, tag="p2")
                    groups = [
                        (p1, w3_ap(var, 0), (-1, 1), False),
                        (p1, w3_ap(var, 1), (0,), True),
                        (p2, w5_ap(img, var, 0), (-2, -1, 1, 2), False),
                        (p2, w5_ap(img, var, 1), (0,), True),
                    ]
                    for ps, wt, shifts, is_last in groups:
                        first = shifts[0] in (-1, -2)
                        for si, sh in enumerate(shifts):
                            for c in (0, 512):
                                mm = nc.tensor.matmul(
                                    ps[0:nout, c:c + 512],
                                    wt[0:k_tot, 0:nout],
                                    xt3[0:k_tot, blk,
                                        2 + sh + c:2 + sh + c + 512],
                                    start=(first and si == 0),
                                    stop=is_last)
                                if first_mm is None:
                                    first_mm = mm

                    # edge mask: nonzero where |lap| > thr
                    a_t = apool.tile([128, 1024], F32, tag="a")
                    em_t = empool.tile([128, 1024], F32, tag="em")
                    nc.scalar.activation(a_t[0:nout, :], p1[0:nout, :],
                                         mybir.ActivationFunctionType.Abs)
                    nc.scalar.activation(em_t[0:nout, :], a_t[0:nout, :],
                                         mybir.ActivationFunctionType.Relu,
                                         bias=nt_ap[0:nout, :])
                    # v = x; v <- sm where edge; out-block = (v > ft)
                    v_t = vpool.tile([128, 1024], F32, tag="v")
                    nc.vector.tensor_copy(v_t[0:nout, :],
                                          xt3[0:nout, blk, 2:1026]
                                          .bitcast(F32))
                    nc.vector.copy_predicated(v_t[0:nout, :],
                                              em_t[0:nout, :]
                                              .bitcast(mybir.dt.int32),
                                              p2[0:nout, 0:1024])
                    ot3 = oa3 if t < 4 else ob3
                    nc.vector.tensor_scalar(ot3[0:nout, blk, :],
                                            v_t[0:nout, :],
                                            ft_ap[0:nout, :], None,
                                            mybir.AluOpType.is_gt)

                    if img == 0 and t == 0:
                        # prefetch image 1 only once image 0's compute has
                        # begun, so its packets don't steal SDMA slots from
                        # the image-0 loads
                        xab[1] = emit_loads(1)
                        add_dep_helper(xab[1][2].ins, first_mm.ins,
                                       reason="delay img1 prefetch")
                    # packed per-half-image stores (big SWDGE transfers)
                    if t == 3:
                        nc.gpsimd.dma_start(
                            out=bass.AP(y, img * H * W,
                                        [[W, 124], [124 * W, 4], [1, W]]),
                            in_=oa3[0:124, 0:4, :])
                    if t == 8:
                        nc.gpsimd.dma_start(
                            out=bass.AP(y, (img * H + 496) * W,
                                        [[W, 124], [124 * W, 4], [1, W]]),
                            in_=ob3[0:124, 0:4, :])
                        nc.gpsimd.dma_start(out=y.ap()[img, 992:1024, :],
                                            in_=ob3[0:32, 4, :])
    nc.compile()
    return nc


def _in_maps(mask, blur_strength, edge_sensitivity, final_threshold):
    mask = np.ascontiguousarray(mask.reshape(16, H, W), np.float32)
    bs = np.asarray(blur_strength, np.float32).reshape(16)
    es = np.asarray(edge_sensitivity, np.float32).reshape(16)
    fts = np.asarray(final_threshold, np.float32).reshape(16)

    w3 = np.zeros((3, 2, 128, 128), np.float32)
    for v, (v3, v5t, ident, k_tot, nout) in enumerate(_TEMPLATES):
        w3[v, 0] = -v3
        w3[v, 1] = 9.0 * ident - v3
    w3p = np.ascontiguousarray(
        w3.transpose(2, 0, 1, 3).reshape(128, 3 * 2 * 128))

    maps = []
    for c in range(N_CORES):
        sel = slice(2 * c, 2 * c + 2)
        w5 = np.zeros((IMGS_PER_CORE, 3, 2, 128, 128), np.float32)
        for i in range(IMGS_PER_CORE):
            bf = bs[2 * c + i] / 3.0
            for v, (v3, v5t, ident, k_tot, nout) in enumerate(_TEMPLATES):
                w5[i, v, 0] = (bf / 25.0) * v5t
                w5[i, v, 1] = (bf / 25.0) * v5t + (1.0 - bf) * ident
        w5p = np.ascontiguousarray(
            w5.transpose(3, 0, 1, 2, 4).reshape(
                128, IMGS_PER_CORE * 3 * 2 * 128))
        negthr = np.zeros((IMGS_PER_CORE, 128, 1), np.float32)
        ftm = np.zeros((IMGS_PER_CORE, 128, 1), np.float32)
        for i in range(IMGS_PER_CORE):
            negthr[i, :, 0] = -(0.5 * es[2 * c + i])
            ftm[i, :, 0] = fts[2 * c + i]
        maps.append({
            "x": np.ascontiguousarray(mask[sel]),
            "w3p": w3p,
            "w5p": w5p,
            "negthr": negthr,
            "ft": ftm,
        })
    return maps


def kernel(mask, blur_strength, edge_sensitivity, final_threshold):
    global _compiled, last_results
    if _compiled is None:
        _compiled = _build()
    maps = _in_maps(mask, blur_strength, edge_sensitivity, final_threshold)
    res = run_bass_kernel_spmd(_compiled, maps, core_ids=list(range(N_CORES)))
    last_results = res
    out = np.empty((16, 1, H, W), np.float32)
    for c in range(N_CORES):
        out[2 * c:2 * c + 2, 0] = res.results[c]["out"]
    return out
